# revision 1
# baseline (speedup 1.0000x reference)
"""Multi-head attention kernel for Trainium2, sharded over 8 NeuronCores.

Problem: q,k,v [2, 32, 2048, 128] f32, mask [2, 1, 2048, 2048] bool.
  out = softmax(q @ k.T / sqrt(128), where(mask)) @ v

Sharding (data + head parallel): core c -> batch c//4, heads (c%4)*8..+8.
Each core computes 8 heads entirely locally.

Per-head device algorithm (T=S=2048, H=128):
  - mm1 computes S^T (s on partitions, t on free) so that the exp'd tiles are
    already in the [s, t] layout that mm2 (O^T = V^T @ P^T) wants as its
    streaming operand -> no on-device transposes of the attention matrix.
  - mm1: psum[s_tile, t_blk] = kT[:, s_tile].T @ qT[:, t_blk]   (float32r)
  - ACT: E = exp(SCALE * psum) -> bf16 SBUF tiles (scale fused into ACT)
  - DVE: E *= maskT tile (bf16 {0,1}; multiplicative mask == additive -inf
    mask because exp(min_f32 + x) == 0 in f32)
  - PE:  psum_o += V[s_tile].T @ E   (O^T unnormalized, accumulated over s)
  - PE:  psum_l += ones.T @ E        (softmax denominator l[t])
  - DVE evacuates psum -> SBUF (DMA cannot access PSUM on TRN2), DMA out.
Host divides O^T by l and transposes back to [t, h] while unsharding.
"""

import sys

try:
    import concourse  # noqa: F401
except ImportError:  # pragma: no cover
    sys.path.insert(0, "/opt/trn_rl_repo")

from contextlib import ExitStack

import numpy as np
import ml_dtypes

import concourse.bacc as bacc
import concourse.tile as tile
from concourse import mybir
from concourse.bass_utils import run_bass_kernel_spmd

N_CORES = 8
B, N, T, S, H = 2, 32, 2048, 2048, 128
HPC = 8  # heads per core
NS = S // 128  # 16 s-tiles
TB = 1024  # t block width (2 psum banks)
NTB = T // TB
MM_N = 512  # matmul free-dim (1 psum bank)
SCALE = 1.0 / np.sqrt(128.0)

_CACHE = {}


def _build(repeat=1, ones_mm=True, mask_mode="pe", exp_act=True, mm2=True,
           qk_dt="f32r"):
    f32 = mybir.dt.float32
    f32r = mybir.dt.float32r
    bf16 = mybir.dt.bfloat16

    nc = bacc.Bacc("TRN2", target_bir_lowering=False, debug=False,
                   num_devices=N_CORES)

    qkd = f32r if qk_dt == "f32r" else bf16
    qT = nc.dram_tensor("qT", [HPC, H, T], qkd, kind="ExternalInput").ap()
    kT = nc.dram_tensor("kT", [HPC, H, S], qkd, kind="ExternalInput").ap()
    v = nc.dram_tensor("v", [HPC, S, H], bf16, kind="ExternalInput").ap()
    # mask^T: "pe" mode = additive {0, -1e38}; "dve" mode = multiplicative {0, 1}
    mT = nc.dram_tensor("mT", [S, T], bf16, kind="ExternalInput").ap()
    ident = nc.dram_tensor("ident", [128, 128], bf16, kind="ExternalInput").ap()
    oT = nc.dram_tensor("oT", [HPC, H, T], f32, kind="ExternalOutput").ap()
    lout = nc.dram_tensor("l", [HPC, T], f32, kind="ExternalOutput").ap()

    with tile.TileContext(nc) as tc, ExitStack() as ctx:
        consts = ctx.enter_context(tc.tile_pool(name="consts", bufs=1))
        qk = ctx.enter_context(tc.tile_pool(name="qk", bufs=2))
        vpool = ctx.enter_context(tc.tile_pool(name="vp", bufs=2))
        epool = ctx.enter_context(tc.tile_pool(name="e", bufs=2 * NS + 2))
        osb = ctx.enter_context(tc.tile_pool(name="osb", bufs=2))
        ps_s = ctx.enter_context(tc.tile_pool(name="ps_s", bufs=2, space="PSUM"))
        ps_o = ctx.enter_context(tc.tile_pool(name="ps_o", bufs=1, space="PSUM"))
        ps_l = ctx.enter_context(tc.tile_pool(name="ps_l", bufs=1, space="PSUM"))

        # mask^T resident for the whole kernel: [128, s_tile, t]
        mask_sb = consts.tile([128, NS, T], bf16)
        nc.sync.dma_start(out=mask_sb, in_=mT.rearrange("(i p) t -> p i t", p=128))
        const_es = None
        if exp_act == "skip":
            const_es = [consts.tile([128, TB], bf16, name=f"ce{i}")
                        for i in range(NS)]
            for t_ in const_es:
                nc.vector.memset(t_, 0.001)
        # full 128-col all-ones stationary: the denominator matmul then has the
        # same weight shape as mm2's V tiles, so PE weight swaps stay in FWL
        # mode (a [128,1] stationary costs ~230ns/swap in mode thrash).
        # Output rows are 128 identical copies of l; we evacuate row 0.
        ones_sb = consts.tile([128, 128], bf16)
        nc.vector.memset(ones_sb, 1.0)
        ident_sb = None
        if mask_mode == "pe":
            ident_sb = consts.tile([128, 128], bf16, name="ident_sb")
            nc.sync.dma_start(out=ident_sb, in_=ident)

        rep_ctx = tc.For_i(0, repeat, 1) if repeat > 1 else None
        if rep_ctx is not None:
            ctx.enter_context(rep_ctx)

        def emit_slot(curr, prev):
            """Emit producer work for `curr` = (h, tb, qT_sb, kT_sb) interleaved
            per s-tile with consumer matmuls for `prev` = (h, tb, es, v_sb, po, pl).
            PE's in-order stream then always has ready consumer MMs to chew on
            while the next producer MM waits for a free mm1-psum slot."""
            es = []
            pe_mask = mask_mode == "pe"
            for si in range(NS):
                if curr is not None:
                    h, tb, qT_sb, kT_sb = curr
                    tsl = slice(tb * TB, (tb + 1) * TB)
                    ps = ps_s.tile([128, TB], mybir.dt.float32, tag="ps", name="ps")
                    ksl = kT_sb[:, si * 128:(si + 1) * 128]
                    for c in range(TB // MM_N):
                        csl = slice(c * MM_N, (c + 1) * MM_N)
                        nc.tensor.matmul(
                            ps[:, csl], ksl,
                            qT_sb[:, tb * TB + c * MM_N: tb * TB + (c + 1) * MM_N],
                            start=True, stop=not pe_mask)
                        if pe_mask:
                            nc.tensor.matmul(
                                ps[:, csl], ident_sb,
                                mask_sb[:, si, tb * TB + c * MM_N: tb * TB + (c + 1) * MM_N],
                                start=False, stop=True)
                    if exp_act == "skip":
                        es.append(const_es[si])
                    else:
                        e = epool.tile([128, TB], bf16, tag="e", name="e")
                        func = (mybir.ActivationFunctionType.Exp if exp_act
                                else mybir.ActivationFunctionType.Copy)
                        nc.scalar.activation(e, ps, func, scale=SCALE)
                        if mask_mode == "dve":
                            nc.vector.tensor_mul(e, e, mask_sb[:, si, tsl])
                        es.append(e)
                if prev is not None and mm2:
                    ph, ptb, pes, pv_sb, po, pl = prev
                    for c in range(TB // MM_N):
                        csl = slice(c * MM_N, (c + 1) * MM_N)
                        nc.tensor.matmul(po[:, csl], pv_sb[:, si, :], pes[si][:, csl],
                                         start=(si == 0), stop=(si == NS - 1))
            if prev is not None and mm2 and ones_mm:
                # dense same-stationary run: 32 ones-matmuls, one weight load;
                # psum-bank alternation without weight swaps is free on PE
                ph, ptb, pes, pv_sb, po, pl = prev
                for si in range(NS):
                    for c in range(TB // MM_N):
                        csl = slice(c * MM_N, (c + 1) * MM_N)
                        nc.tensor.matmul(pl[:, csl], ones_sb, pes[si][:, csl],
                                         start=(si == 0), stop=(si == NS - 1))
                        # (pl rows are all identical; row 0 is read out)
            if prev is not None:
                ph, ptb, pes, pv_sb, po, pl = prev
                ptsl = slice(ptb * TB, (ptb + 1) * TB)
                o_sb = osb.tile([H, TB], mybir.dt.float32, tag="o", name="o_sb")
                if mm2:
                    nc.vector.tensor_copy(o_sb, po)
                else:
                    for si in range(NS):
                        nc.vector.tensor_copy(o_sb[:, si * 8:(si + 1) * 8],
                                              pes[si][:, :8])
                nc.sync.dma_start(out=oT[ph][:, ptsl], in_=o_sb)
                if ones_mm and mm2:
                    l_sb = osb.tile([1, TB], mybir.dt.float32, tag="l", name="l_sb")
                    nc.vector.tensor_copy(l_sb, pl[0:1, :])
                    nc.sync.dma_start(out=lout[ph:ph + 1, ptsl], in_=l_sb)
            return es

        def mk_prev(h, tb, es, v_sb):
            po = pl = None
            if mm2:
                po = ps_o.tile([H, TB], mybir.dt.float32, tag="po", name="po")
                if ones_mm:
                    pl = ps_l.tile([H, TB], mybir.dt.float32, tag="pl", name="pl")
            return (h, tb, es, v_sb, po, pl)

        pending = None
        for h in range(HPC):
            qT_sb = qk.tile([H, T], qkd, tag="q", name="qT_sb")
            nc.sync.dma_start(out=qT_sb, in_=qT[h])
            kT_sb = qk.tile([H, S], qkd, tag="k", name="kT_sb")
            nc.sync.dma_start(out=kT_sb, in_=kT[h])
            v_sb = vpool.tile([128, NS, H], bf16, tag="v", name="v_sb")
            nc.sync.dma_start(out=v_sb, in_=v[h].rearrange("(i p) d -> p i d", p=128))
            for tb in range(NTB):
                es = emit_slot((h, tb, qT_sb, kT_sb), pending)
                pending = mk_prev(h, tb, es, v_sb)
        emit_slot(None, pending)

    nc.compile()
    return nc


def _build_v4(repeat=1, ones_mm=True, exp_act=True, qk_dt="f32r"):
    """v4: all matmuls in >=4-instruction same-stationary runs.

    PE microbenchmarks show a same-stationary N=512 matmul costs ~117ns, but
    alternating stationary + psum-target every 1-2 matmuls costs ~230ns; at
    groups of 4 the swap overhead mostly vanishes.  So:
      - mm1 is emitted si-outer: one kT[:,si] weight load covers the full
        t=2048 row (4 matmuls into a [128, 2048] 4-bank psum tile).
      - mm2 pairs s-tiles (2 weight loads, then 4 ones-matmuls in one run).
    Pipeline slots are half-heads; the consumer chain for (h, tb0) runs
    during slot (h, 1) and (h, tb1) during slot (h+1, 0), so consumer MMs
    always read exp outputs that are already (or just) materialized while
    mm1 of the current slot trickles at ACT's pace.
    PSUM: ps [128,2048] x1 (4 banks) + po (2) + pl (2) = 8.
    """
    f32 = mybir.dt.float32
    f32r = mybir.dt.float32r
    bf16 = mybir.dt.bfloat16
    qkd = f32r if qk_dt == "f32r" else bf16

    nc = bacc.Bacc("TRN2", target_bir_lowering=False, debug=False,
                   num_devices=N_CORES)
    qT = nc.dram_tensor("qT", [HPC, H, T], qkd, kind="ExternalInput").ap()
    kT = nc.dram_tensor("kT", [HPC, H, S], qkd, kind="ExternalInput").ap()
    v = nc.dram_tensor("v", [HPC, S, H], bf16, kind="ExternalInput").ap()
    mT = nc.dram_tensor("mT", [S, T], bf16, kind="ExternalInput").ap()
    oT = nc.dram_tensor("oT", [HPC, H, T], f32, kind="ExternalOutput").ap()
    lout = nc.dram_tensor("l", [HPC, T], f32, kind="ExternalOutput").ap()

    with tile.TileContext(nc) as tc, ExitStack() as ctx:
        consts = ctx.enter_context(tc.tile_pool(name="consts", bufs=1))
        qk = ctx.enter_context(tc.tile_pool(name="qk", bufs=2))
        vpool = ctx.enter_context(tc.tile_pool(name="vp", bufs=3))
        epool = ctx.enter_context(tc.tile_pool(name="e", bufs=36))
        osb = ctx.enter_context(tc.tile_pool(name="osb", bufs=2))
        ps_s = ctx.enter_context(tc.tile_pool(name="ps_s", bufs=1, space="PSUM"))
        ps_o = ctx.enter_context(tc.tile_pool(name="ps_o", bufs=1, space="PSUM"))
        ps_l = ctx.enter_context(tc.tile_pool(name="ps_l", bufs=1, space="PSUM"))

        mask_sb = consts.tile([128, NS, T], bf16)
        nc.sync.dma_start(out=mask_sb, in_=mT.rearrange("(i p) t -> p i t", p=128))
        ones_sb = consts.tile([128, 128], bf16)
        nc.vector.memset(ones_sb, 1.0)

        rep_ctx = tc.For_i(0, repeat, 1) if repeat > 1 else None
        if rep_ctx is not None:
            ctx.enter_context(rep_ctx)

        # E tiles: dict (h % 2, tb, si) -> tile
        etiles = {}

        def produce(h, half, sj, qT_sb, kT_sb):
            si = half * 8 + sj
            ps = ps_s.tile([128, T], mybir.dt.float32, tag="ps", name="ps")
            ksl = kT_sb[:, si * 128:(si + 1) * 128]
            for c in range(T // MM_N):  # 4 matmuls, one weight load
                csl = slice(c * MM_N, (c + 1) * MM_N)
                nc.tensor.matmul(ps[:, csl], ksl, qT_sb[:, csl],
                                 start=True, stop=True)
            func = (mybir.ActivationFunctionType.Exp if exp_act
                    else mybir.ActivationFunctionType.Copy)
            for tb in range(NTB):
                e = epool.tile([128, TB], bf16, tag="e", name="e")
                nc.scalar.activation(e, ps[:, tb * TB:(tb + 1) * TB], func,
                                     scale=SCALE)
                nc.vector.tensor_mul(e, e, mask_sb[:, si, tb * TB:(tb + 1) * TB])
                etiles[(h % 2, tb, si)] = e

        def consume(ch, ctb, sj, v_sb, po, pl):
            """Consumer work for si pair (2sj, 2sj+1) of chain (ch, ctb)."""
            s0, s1 = 2 * sj, 2 * sj + 1
            for si in (s0, s1):
                e = etiles[(ch % 2, ctb, si)]
                for c in range(TB // MM_N):
                    csl = slice(c * MM_N, (c + 1) * MM_N)
                    nc.tensor.matmul(po[:, csl], v_sb[:, si, :], e[:, csl],
                                     start=(si == 0), stop=(si == NS - 1))
            if ones_mm:
                for si in (s0, s1):
                    e = etiles[(ch % 2, ctb, si)]
                    for c in range(TB // MM_N):
                        csl = slice(c * MM_N, (c + 1) * MM_N)
                        nc.tensor.matmul(pl[:, csl], ones_sb, e[:, csl],
                                         start=(si == 0), stop=(si == NS - 1))

        def writeback(ch, ctb, po, pl):
            ptsl = slice(ctb * TB, (ctb + 1) * TB)
            o_sb = osb.tile([H, TB], mybir.dt.float32, tag="o", name="o_sb")
            nc.vector.tensor_copy(o_sb, po)
            nc.sync.dma_start(out=oT[ch][:, ptsl], in_=o_sb)
            if ones_mm:
                l_sb = osb.tile([1, TB], mybir.dt.float32, tag="l", name="l_sb")
                nc.vector.tensor_copy(l_sb, pl[0:1, :])
                nc.sync.dma_start(out=lout[ch:ch + 1, ptsl], in_=l_sb)

        vtiles = {}
        pending = None  # (ch, ctb)
        for h in range(HPC):
            qT_sb = qk.tile([H, T], qkd, tag="q", name="qT_sb")
            nc.sync.dma_start(out=qT_sb, in_=qT[h])
            kT_sb = qk.tile([H, S], qkd, tag="k", name="kT_sb")
            nc.sync.dma_start(out=kT_sb, in_=kT[h])
            v_sb = vpool.tile([128, NS, H], bf16, tag="v", name="v_sb")
            nc.sync.dma_start(out=v_sb, in_=v[h].rearrange("(i p) d -> p i d", p=128))
            vtiles[h % 2] = v_sb
            for half in range(2):
                # consumer chain for this slot
                if half == 1:
                    cons = (h, 0)
                else:
                    cons = (h - 1, 1) if h > 0 else None
                po = pl = None
                if cons is not None:
                    po = ps_o.tile([H, TB], mybir.dt.float32, tag="po", name="po")
                    if ones_mm:
                        pl = ps_l.tile([H, TB], mybir.dt.float32, tag="pl", name="pl")
                for sj in range(8):
                    produce(h, half, sj, qT_sb, kT_sb)
                    if cons is not None:
                        consume(cons[0], cons[1], sj, vtiles[cons[0] % 2], po, pl)
                if cons is not None:
                    writeback(cons[0], cons[1], po, pl)
        # flush last chain: (HPC-1, tb1)
        po = ps_o.tile([H, TB], mybir.dt.float32, tag="po", name="po")
        pl = None
        if ones_mm:
            pl = ps_l.tile([H, TB], mybir.dt.float32, tag="pl", name="pl")
        for sj in range(8):
            consume(HPC - 1, 1, sj, vtiles[(HPC - 1) % 2], po, pl)
        writeback(HPC - 1, 1, po, pl)

    nc.compile()
    return nc


def _get_nc():
    if "nc" not in _CACHE:
        _CACHE["nc"] = _build()
    return _CACHE["nc"]


def _shard_inputs(q, k, v, mask, mask_mode="pe", qk_dt="f32r"):
    bf16 = ml_dtypes.bfloat16
    in_maps = []
    maskT = {}
    for b in range(B):
        mt = np.ascontiguousarray(mask[b, 0].T)
        if mask_mode == "pe":
            # additive bias: 0 where unmasked, -1e38 where masked
            maskT[b] = np.where(mt, np.float32(0.0),
                                np.float32(-1e38)).astype(bf16)
        else:
            maskT[b] = mt.astype(bf16)
    ident = np.eye(128, dtype=bf16)
    for c in range(N_CORES):
        b = c // 4
        h0 = (c % 4) * HPC
        in_maps.append({
            "qT": np.ascontiguousarray(
                q[b, h0:h0 + HPC].transpose(0, 2, 1)).astype(
                    np.float32 if qk_dt == "f32r" else bf16),
            "kT": np.ascontiguousarray(
                k[b, h0:h0 + HPC].transpose(0, 2, 1)).astype(
                    np.float32 if qk_dt == "f32r" else bf16),
            "v": v[b, h0:h0 + HPC].astype(bf16),
            "mT": maskT[b],
            "ident": ident,
        })
    return in_maps


def kernel(q, k, v, mask):
    nc = _get_nc()
    in_maps = _shard_inputs(q, k, v, mask)
    res = run_bass_kernel_spmd(nc, in_maps, list(range(N_CORES)))
    out = np.empty((B, N, T, H), dtype=np.float32)
    for c in range(N_CORES):
        b = c // 4
        h0 = (c % 4) * HPC
        oT_c = res.results[c]["oT"]  # [HPC, H, T] unnormalized
        l_c = res.results[c]["l"]    # [HPC, T]
        out[b, h0:h0 + HPC] = (oT_c / l_c[:, None, :]).transpose(0, 2, 1)
    return out



# revision 23
# speedup vs baseline: 482.2131x; 482.2131x over previous
"""Multi-head attention kernel for Trainium2, sharded over 8 NeuronCores.

Problem: q,k,v [2, 32, 2048, 128] f32, mask [2, 1, 2048, 2048] bool.
  out = softmax(q @ k.T / sqrt(128), where(mask)) @ v

Sharding (data + head parallel): core c -> batch c//4, heads (c%4)*8..+8.
Each core computes 8 heads entirely locally.

Per-head device algorithm (T=S=2048, H=128):
  - mm1 computes S^T (s on partitions, t on free) so that the exp'd tiles are
    already in the [s, t] layout that mm2 (O^T = V^T @ P^T) wants as its
    streaming operand -> no on-device transposes of the attention matrix.
  - mm1: psum[s_tile, t_blk] = kT[:, s_tile].T @ qT[:, t_blk]   (float32r)
  - ACT: E = exp(SCALE * psum) -> bf16 SBUF tiles (scale fused into ACT)
  - DVE: E *= maskT tile (bf16 {0,1}; multiplicative mask == additive -inf
    mask because exp(min_f32 + x) == 0 in f32)
  - PE:  psum_o += V[s_tile].T @ E   (O^T unnormalized, accumulated over s)
  - PE:  psum_l += ones.T @ E        (softmax denominator l[t])
  - DVE evacuates psum -> SBUF (DMA cannot access PSUM on TRN2), DMA out.
Host divides O^T by l and transposes back to [t, h] while unsharding.
"""

import sys

try:
    import concourse  # noqa: F401
except ImportError:  # pragma: no cover
    sys.path.insert(0, "/opt/trn_rl_repo")

from contextlib import ExitStack

import numpy as np
import ml_dtypes

import concourse.bacc as bacc
import concourse.tile as tile
from concourse import mybir
from concourse.bass_utils import run_bass_kernel_spmd

N_CORES = 8
B, N, T, S, H = 2, 32, 2048, 2048, 128
HPC = 8  # heads per core
NS = S // 128  # 16 s-tiles
TB = 1024  # t block width (2 psum banks)
NTB = T // TB
MM_N = 512  # matmul free-dim (1 psum bank)
SCALE = 1.0 / np.sqrt(128.0)

_CACHE = {}


def _register_dve_exp():
    """Register two chained custom DVE ops computing exp(x)*mask.

    op1 EXP_LADDER_SEED: u = x*s0; p = ((s1*u + imm2)*u + 1)*u + 1  (cubic
      Taylor of e^u); out = p^2.  With s0=1/32, s1=1/6, imm2=0.5 this is
      e^(x/16) to ~6e-5 rel.  8 ALU stages (the v3 budget), f32.
    op2 EXP_LADDER_FIN: out = (in0^16) * in1 — four squarings finish the
      ladder (e^x) and fuse the multiplicative mask.  5 stages.

    Registration is process-local: appends to dve_ops.OPS and the name->row
    table, exactly what a source-level op addition would do.  The per-NEFF
    DVE table is generated from these at compile-bir time.
    """
    if "seed" in _CACHE.get("dveops", {}):
        return _CACHE["dveops"]
    import numpy as np
    import concourse.dve_ops as dops
    from concourse.dve_spec import Spec, Src0, Src1, C0, C1, C2, One, sq, lower
    from concourse.dve_spec import _has_src1 as has_src1
    from concourse.dve_uop import DveOpSpec

    def ref_seed(in0, in1, s0, s1, imm2):
        u = in0.astype(np.float32) * np.float32(s0)
        p = ((np.float32(s1) * u + np.float32(imm2)) * u + 1.0) * u + 1.0
        return (p * p).astype(np.float32)

    def ref_fin(in0, in1, s0, s1, imm2):
        x = in0.astype(np.float32)
        for _ in range(4):
            x = x * x
        return (x * in1).astype(np.float32)

    u = Src0 * C0
    specs = [
        ("EXP_LADDER_SEED_ANT",
         Spec(body=sq(((C1 * u + C2) * u + One) * u + One), reference=ref_seed)),
        ("EXP_LADDER_FIN_ANT",
         Spec(body=sq(sq(sq(sq(Src0)))) * Src1, reference=ref_fin)),
    ]
    out = {}
    for name, spec in specs:
        if name not in dops._SUB_OPCODE_FOR_NAME:
            row = max(dops._SUB_OPCODE_FOR_NAME.values()) + 1
            assert row < 0x20
            dops._SUB_OPCODE_FOR_NAME[name] = row
            shas = {}
            for ver in ("v3", "v4"):
                try:
                    s = DveOpSpec(name=name, opcode=row,
                                  uops=lower(spec, ver=ver),
                                  rd1_en=has_src1(spec))
                    shas[ver] = s.sha(ver)
                except Exception:
                    pass
            op = dops.DveOp(name, spec, subdim=False, uops_sha=shas)
            dops.OPS.append(op)
            dops.CUSTOM_DVE_SPECS[name] = spec
        else:
            op = next(o for o in dops.OPS if o.name == name)
        out["seed" if "SEED" in name else "fin"] = op
    _CACHE["dveops"] = out
    return out


def _build(repeat=1, ones_mm=True, mask_mode="pe", exp_act=True, mm2=True,
           qk_dt="f32r"):
    f32 = mybir.dt.float32
    f32r = mybir.dt.float32r
    bf16 = mybir.dt.bfloat16

    nc = bacc.Bacc("TRN2", target_bir_lowering=False, debug=False,
                   num_devices=N_CORES)

    qkd = f32r if qk_dt == "f32r" else bf16
    qT = nc.dram_tensor("qT", [HPC, H, T], qkd, kind="ExternalInput").ap()
    kT = nc.dram_tensor("kT", [HPC, H, S], qkd, kind="ExternalInput").ap()
    v = nc.dram_tensor("v", [HPC, S, H], bf16, kind="ExternalInput").ap()
    # mask^T: "pe" mode = additive {0, -1e38}; "dve" mode = multiplicative {0, 1}
    mT = nc.dram_tensor("mT", [S, T], bf16, kind="ExternalInput").ap()
    ident = nc.dram_tensor("ident", [128, 128], bf16, kind="ExternalInput").ap()
    oT = nc.dram_tensor("oT", [HPC, H, T], f32, kind="ExternalOutput").ap()
    lout = nc.dram_tensor("l", [HPC, T], f32, kind="ExternalOutput").ap()

    with tile.TileContext(nc) as tc, ExitStack() as ctx:
        consts = ctx.enter_context(tc.tile_pool(name="consts", bufs=1))
        qk = ctx.enter_context(tc.tile_pool(name="qk", bufs=2))
        vpool = ctx.enter_context(tc.tile_pool(name="vp", bufs=2))
        epool = ctx.enter_context(tc.tile_pool(name="e", bufs=2 * NS + 2))
        osb = ctx.enter_context(tc.tile_pool(name="osb", bufs=2))
        ps_s = ctx.enter_context(tc.tile_pool(name="ps_s", bufs=2, space="PSUM"))
        ps_o = ctx.enter_context(tc.tile_pool(name="ps_o", bufs=1, space="PSUM"))
        ps_l = ctx.enter_context(tc.tile_pool(name="ps_l", bufs=1, space="PSUM"))

        # mask^T resident for the whole kernel: [128, s_tile, t]
        mask_sb = consts.tile([128, NS, T], bf16)
        nc.sync.dma_start(out=mask_sb, in_=mT.rearrange("(i p) t -> p i t", p=128))
        const_es = None
        if exp_act == "skip":
            const_es = [consts.tile([128, TB], bf16, name=f"ce{i}")
                        for i in range(NS)]
            for t_ in const_es:
                nc.vector.memset(t_, 0.001)
        # full 128-col all-ones stationary: the denominator matmul then has the
        # same weight shape as mm2's V tiles, so PE weight swaps stay in FWL
        # mode (a [128,1] stationary costs ~230ns/swap in mode thrash).
        # Output rows are 128 identical copies of l; we evacuate row 0.
        ones_sb = consts.tile([128, 128], bf16)
        nc.vector.memset(ones_sb, 1.0)
        ident_sb = None
        if mask_mode == "pe":
            ident_sb = consts.tile([128, 128], bf16, name="ident_sb")
            nc.sync.dma_start(out=ident_sb, in_=ident)

        rep_ctx = tc.For_i(0, repeat, 1) if repeat > 1 else None
        if rep_ctx is not None:
            ctx.enter_context(rep_ctx)

        def emit_slot(curr, prev):
            """Emit producer work for `curr` = (h, tb, qT_sb, kT_sb) interleaved
            per s-tile with consumer matmuls for `prev` = (h, tb, es, v_sb, po, pl).
            PE's in-order stream then always has ready consumer MMs to chew on
            while the next producer MM waits for a free mm1-psum slot."""
            es = []
            pe_mask = mask_mode == "pe"
            for si in range(NS):
                if curr is not None:
                    h, tb, qT_sb, kT_sb = curr
                    tsl = slice(tb * TB, (tb + 1) * TB)
                    ps = ps_s.tile([128, TB], mybir.dt.float32, tag="ps", name="ps")
                    ksl = kT_sb[:, si * 128:(si + 1) * 128]
                    for c in range(TB // MM_N):
                        csl = slice(c * MM_N, (c + 1) * MM_N)
                        nc.tensor.matmul(
                            ps[:, csl], ksl,
                            qT_sb[:, tb * TB + c * MM_N: tb * TB + (c + 1) * MM_N],
                            start=True, stop=not pe_mask)
                        if pe_mask:
                            nc.tensor.matmul(
                                ps[:, csl], ident_sb,
                                mask_sb[:, si, tb * TB + c * MM_N: tb * TB + (c + 1) * MM_N],
                                start=False, stop=True)
                    if exp_act == "skip":
                        es.append(const_es[si])
                    else:
                        e = epool.tile([128, TB], bf16, tag="e", name="e")
                        func = (mybir.ActivationFunctionType.Exp if exp_act
                                else mybir.ActivationFunctionType.Copy)
                        nc.scalar.activation(e, ps, func, scale=SCALE)
                        if mask_mode == "dve":
                            nc.vector.tensor_mul(e, e, mask_sb[:, si, tsl])
                        es.append(e)
                if prev is not None and mm2:
                    ph, ptb, pes, pv_sb, po, pl = prev
                    for c in range(TB // MM_N):
                        csl = slice(c * MM_N, (c + 1) * MM_N)
                        nc.tensor.matmul(po[:, csl], pv_sb[:, si, :], pes[si][:, csl],
                                         start=(si == 0), stop=(si == NS - 1))
            if prev is not None and mm2 and ones_mm:
                # dense same-stationary run: 32 ones-matmuls, one weight load;
                # psum-bank alternation without weight swaps is free on PE
                ph, ptb, pes, pv_sb, po, pl = prev
                for si in range(NS):
                    for c in range(TB // MM_N):
                        csl = slice(c * MM_N, (c + 1) * MM_N)
                        nc.tensor.matmul(pl[:, csl], ones_sb, pes[si][:, csl],
                                         start=(si == 0), stop=(si == NS - 1))
                        # (pl rows are all identical; row 0 is read out)
            if prev is not None:
                ph, ptb, pes, pv_sb, po, pl = prev
                ptsl = slice(ptb * TB, (ptb + 1) * TB)
                o_sb = osb.tile([H, TB], mybir.dt.float32, tag="o", name="o_sb")
                if mm2:
                    nc.vector.tensor_copy(o_sb, po)
                else:
                    for si in range(NS):
                        nc.vector.tensor_copy(o_sb[:, si * 8:(si + 1) * 8],
                                              pes[si][:, :8])
                nc.sync.dma_start(out=oT[ph][:, ptsl], in_=o_sb)
                if ones_mm and mm2:
                    l_sb = osb.tile([1, TB], mybir.dt.float32, tag="l", name="l_sb")
                    nc.vector.tensor_copy(l_sb, pl[0:1, :])
                    nc.sync.dma_start(out=lout[ph:ph + 1, ptsl], in_=l_sb)
            return es

        def mk_prev(h, tb, es, v_sb):
            po = pl = None
            if mm2:
                po = ps_o.tile([H, TB], mybir.dt.float32, tag="po", name="po")
                if ones_mm:
                    pl = ps_l.tile([H, TB], mybir.dt.float32, tag="pl", name="pl")
            return (h, tb, es, v_sb, po, pl)

        pending = None
        for h in range(HPC):
            qT_sb = qk.tile([H, T], qkd, tag="q", name="qT_sb")
            nc.sync.dma_start(out=qT_sb, in_=qT[h])
            kT_sb = qk.tile([H, S], qkd, tag="k", name="kT_sb")
            nc.sync.dma_start(out=kT_sb, in_=kT[h])
            v_sb = vpool.tile([128, NS, H], bf16, tag="v", name="v_sb")
            nc.sync.dma_start(out=v_sb, in_=v[h].rearrange("(i p) d -> p i d", p=128))
            for tb in range(NTB):
                es = emit_slot((h, tb, qT_sb, kT_sb), pending)
                pending = mk_prev(h, tb, es, v_sb)
        emit_slot(None, pending)

    nc.compile()
    return nc


def _build_v4(repeat=1, ones_mm=True, exp_act=True, qk_dt="f32r"):
    """v4: all matmuls in >=4-instruction same-stationary runs.

    PE microbenchmarks show a same-stationary N=512 matmul costs ~117ns, but
    alternating stationary + psum-target every 1-2 matmuls costs ~230ns; at
    groups of 4 the swap overhead mostly vanishes.  So:
      - mm1 is emitted si-outer: one kT[:,si] weight load covers the full
        t=2048 row (4 matmuls into a [128, 2048] 4-bank psum tile).
      - mm2 pairs s-tiles (2 weight loads, then 4 ones-matmuls in one run).
    Pipeline slots are half-heads; the consumer chain for (h, tb0) runs
    during slot (h, 1) and (h, tb1) during slot (h+1, 0), so consumer MMs
    always read exp outputs that are already (or just) materialized while
    mm1 of the current slot trickles at ACT's pace.
    PSUM: ps [128,2048] x1 (4 banks) + po (2) + pl (2) = 8.
    """
    f32 = mybir.dt.float32
    f32r = mybir.dt.float32r
    bf16 = mybir.dt.bfloat16
    qkd = f32r if qk_dt == "f32r" else bf16

    nc = bacc.Bacc("TRN2", target_bir_lowering=False, debug=False,
                   num_devices=N_CORES)
    qT = nc.dram_tensor("qT", [HPC, H, T], qkd, kind="ExternalInput").ap()
    kT = nc.dram_tensor("kT", [HPC, H, S], qkd, kind="ExternalInput").ap()
    v = nc.dram_tensor("v", [HPC, S, H], bf16, kind="ExternalInput").ap()
    mT = nc.dram_tensor("mT", [S, T], bf16, kind="ExternalInput").ap()
    oT = nc.dram_tensor("oT", [HPC, H, T], f32, kind="ExternalOutput").ap()
    lout = nc.dram_tensor("l", [HPC, T], f32, kind="ExternalOutput").ap()

    with tile.TileContext(nc) as tc, ExitStack() as ctx:
        consts = ctx.enter_context(tc.tile_pool(name="consts", bufs=1))
        qk = ctx.enter_context(tc.tile_pool(name="qk", bufs=2))
        vpool = ctx.enter_context(tc.tile_pool(name="vp", bufs=3))
        epool = ctx.enter_context(tc.tile_pool(name="e", bufs=36))
        osb = ctx.enter_context(tc.tile_pool(name="osb", bufs=2))
        ps_s = ctx.enter_context(tc.tile_pool(name="ps_s", bufs=1, space="PSUM"))
        ps_o = ctx.enter_context(tc.tile_pool(name="ps_o", bufs=1, space="PSUM"))
        ps_l = ctx.enter_context(tc.tile_pool(name="ps_l", bufs=1, space="PSUM"))

        mask_sb = consts.tile([128, NS, T], bf16)
        nc.sync.dma_start(out=mask_sb, in_=mT.rearrange("(i p) t -> p i t", p=128))
        ones_sb = consts.tile([128, 128], bf16)
        nc.vector.memset(ones_sb, 1.0)

        rep_ctx = tc.For_i(0, repeat, 1) if repeat > 1 else None
        if rep_ctx is not None:
            ctx.enter_context(rep_ctx)

        # E tiles: dict (h % 2, tb, si) -> tile
        etiles = {}

        def produce(h, half, sj, qT_sb, kT_sb):
            si = half * 8 + sj
            ps = ps_s.tile([128, T], mybir.dt.float32, tag="ps", name="ps")
            ksl = kT_sb[:, si * 128:(si + 1) * 128]
            for c in range(T // MM_N):  # 4 matmuls, one weight load
                csl = slice(c * MM_N, (c + 1) * MM_N)
                nc.tensor.matmul(ps[:, csl], ksl, qT_sb[:, csl],
                                 start=True, stop=True)
            func = (mybir.ActivationFunctionType.Exp if exp_act
                    else mybir.ActivationFunctionType.Copy)
            for tb in range(NTB):
                e = epool.tile([128, TB], bf16, tag="e", name="e")
                nc.scalar.activation(e, ps[:, tb * TB:(tb + 1) * TB], func,
                                     scale=SCALE)
                nc.vector.tensor_mul(e, e, mask_sb[:, si, tb * TB:(tb + 1) * TB])
                etiles[(h % 2, tb, si)] = e

        def consume(ch, ctb, sj, v_sb, po, pl):
            """Consumer work for si pair (2sj, 2sj+1) of chain (ch, ctb)."""
            s0, s1 = 2 * sj, 2 * sj + 1
            for si in (s0, s1):
                e = etiles[(ch % 2, ctb, si)]
                for c in range(TB // MM_N):
                    csl = slice(c * MM_N, (c + 1) * MM_N)
                    nc.tensor.matmul(po[:, csl], v_sb[:, si, :], e[:, csl],
                                     start=(si == 0), stop=(si == NS - 1))
            if ones_mm:
                for si in (s0, s1):
                    e = etiles[(ch % 2, ctb, si)]
                    for c in range(TB // MM_N):
                        csl = slice(c * MM_N, (c + 1) * MM_N)
                        nc.tensor.matmul(pl[:, csl], ones_sb, e[:, csl],
                                         start=(si == 0), stop=(si == NS - 1))

        def writeback(ch, ctb, po, pl):
            ptsl = slice(ctb * TB, (ctb + 1) * TB)
            o_sb = osb.tile([H, TB], mybir.dt.float32, tag="o", name="o_sb")
            nc.vector.tensor_copy(o_sb, po)
            nc.sync.dma_start(out=oT[ch][:, ptsl], in_=o_sb)
            if ones_mm:
                l_sb = osb.tile([1, TB], mybir.dt.float32, tag="l", name="l_sb")
                nc.vector.tensor_copy(l_sb, pl[0:1, :])
                nc.sync.dma_start(out=lout[ch:ch + 1, ptsl], in_=l_sb)

        vtiles = {}
        pending = None  # (ch, ctb)
        for h in range(HPC):
            qT_sb = qk.tile([H, T], qkd, tag="q", name="qT_sb")
            nc.sync.dma_start(out=qT_sb, in_=qT[h])
            kT_sb = qk.tile([H, S], qkd, tag="k", name="kT_sb")
            nc.sync.dma_start(out=kT_sb, in_=kT[h])
            v_sb = vpool.tile([128, NS, H], bf16, tag="v", name="v_sb")
            nc.sync.dma_start(out=v_sb, in_=v[h].rearrange("(i p) d -> p i d", p=128))
            vtiles[h % 2] = v_sb
            for half in range(2):
                # consumer chain for this slot
                if half == 1:
                    cons = (h, 0)
                else:
                    cons = (h - 1, 1) if h > 0 else None
                po = pl = None
                if cons is not None:
                    po = ps_o.tile([H, TB], mybir.dt.float32, tag="po", name="po")
                    if ones_mm:
                        pl = ps_l.tile([H, TB], mybir.dt.float32, tag="pl", name="pl")
                for sj in range(8):
                    produce(h, half, sj, qT_sb, kT_sb)
                    if cons is not None:
                        consume(cons[0], cons[1], sj, vtiles[cons[0] % 2], po, pl)
                if cons is not None:
                    writeback(cons[0], cons[1], po, pl)
        # flush last chain: (HPC-1, tb1)
        po = ps_o.tile([H, TB], mybir.dt.float32, tag="po", name="po")
        pl = None
        if ones_mm:
            pl = ps_l.tile([H, TB], mybir.dt.float32, tag="pl", name="pl")
        for sj in range(8):
            consume(HPC - 1, 1, sj, vtiles[(HPC - 1) % 2], po, pl)
        writeback(HPC - 1, 1, po, pl)

    nc.compile()
    return nc


def _build_v5(repeat=1, normalize="dve", skip_b=False, skip_exp=False,
              dve_sis=(), pool_sis=()):
    """v5: flipped mm2 with fused softmax denominator.

    mm1 (per s-tile si, t-half th): psum[s128, t1024] = kT[:,si].T @ qT[:,th]
    ACT: E = exp(SCALE * psum) -> bf16 SBUF [128, 1024]
    DVE: E *= maskT slice ({0,1} bf16, 4x mode)
    mm2 FLIPPED: stationary = E[si][:, t-tile(128)] (bf16), moving =
      [V_si | ones] (128 x 129 bf16)  ->  out[t128, 129] accumulated over si
      in PSUM. Column 128 of the output IS the softmax denominator l[t]:
      the ones-matmul pass of v1 becomes one extra moving column (0.8%).
    DVE: linv = 1/l, o = out[:, :128] * linv (per-partition broadcast),
      DMA out in natural [T, H] layout. No host-side transpose or divide.

    PSUM: mm1 ps 2x[128,1024] (4 banks) + 4 accum bank-tiles [128,512]
    (3 accumulators of width 129 packed per bank at offsets 0/129/258;
    a slot of 8 t-tiles uses 3 bank-tiles, 4 bufs rotate across slots).

    Slot = (head, t-half). B-work (flipped mm2 + evac) of slot k is emitted
    interleaved with A-work (mm1/exp/mask) of slot k+1, so PE fills ACT-wait
    bubbles with consumer matmuls and E tiles die one slot after birth.
    """
    f32 = mybir.dt.float32
    f32r = mybir.dt.float32r
    bf16 = mybir.dt.bfloat16

    TBH = 1024          # t-half width (ACT instruction size)
    NTT = TBH // 128    # t-tiles per slot = 8
    dveops = _register_dve_exp() if dve_sis else None

    nc = bacc.Bacc("TRN2", target_bir_lowering=False, debug=False,
                   num_devices=N_CORES)
    OW = H if normalize == "dve" else H + 1
    qT = nc.dram_tensor("qT", [HPC, H, T], f32r, kind="ExternalInput").ap()
    kT = nc.dram_tensor("kT", [HPC, H, S], f32r, kind="ExternalInput").ap()
    vone = nc.dram_tensor("vone", [HPC, S, H + 1], bf16,
                          kind="ExternalInput").ap()
    mT = nc.dram_tensor("mT", [S, T], bf16, kind="ExternalInput").ap()
    o = nc.dram_tensor("o", [HPC, T, OW], f32, kind="ExternalOutput").ap()

    with tile.TileContext(nc) as tc, ExitStack() as ctx:
        consts = ctx.enter_context(tc.tile_pool(name="consts", bufs=1))
        qk = ctx.enter_context(tc.tile_pool(name="qk", bufs=2))
        vpool = ctx.enter_context(tc.tile_pool(name="vp", bufs=2))
        epool = ctx.enter_context(tc.tile_pool(name="e", bufs=36))
        xpool = (ctx.enter_context(tc.tile_pool(name="x", bufs=3))
                 if dve_sis else None)
        osb = ctx.enter_context(tc.tile_pool(name="osb", bufs=4))
        ps_s = ctx.enter_context(tc.tile_pool(name="ps_s", bufs=2, space="PSUM"))
        ps_a = ctx.enter_context(tc.tile_pool(name="ps_a", bufs=4, space="PSUM"))

        mask_sb = consts.tile([128, NS, T], bf16)
        nc.sync.dma_start(out=mask_sb, in_=mT.rearrange("(i p) t -> p i t", p=128))
        const_e = None
        if skip_exp:
            const_e = consts.tile([128, TBH], bf16, name="const_e")
            nc.vector.memset(const_e, 0.001)

        rep_ctx = tc.For_i(0, repeat, 1) if repeat > 1 else None
        if rep_ctx is not None:
            ctx.enter_context(rep_ctx)

        etiles = {}   # (slot_idx % 2, si) -> E piece [128, TBH]
        vtiles = {}   # h % 2 -> vone_sb
        baccs = {}    # slot_idx -> 3 accumulator bank-tiles (lazy at si==0)
        SLOTS = [(h, th) for h in range(HPC) for th in range(2)]

        def acc_ap(accs, t):
            """Accumulator AP for t-tile t: bank-tile t//3, 8B-aligned offset
            132*(t%3) (129 wide; 132-stride keeps PSUM cachelines aligned)."""
            return accs[t // 3][:, 132 * (t % 3): 132 * (t % 3) + 129]

        def b_mms(slot_idx, si):
            h, th = SLOTS[slot_idx]
            if si == 0:
                baccs[slot_idx] = [
                    ps_a.tile([128, 512], mybir.dt.float32, tag="acc",
                              name="acc") for _ in range(3)]
            accs = baccs[slot_idx]
            e = etiles[(slot_idx % 2, si)]
            v_sb = vtiles[h % 2]
            for t in range(NTT):
                # start=True clears has_written for the WHOLE bank, so only
                # the bank's first accumulator (t%3==0) may use it; the
                # siblings' first matmuls rely on the cleared bits to
                # overwrite (has_written=0 -> overwrite+set).  Emission is t-
                # ascending, so the clear lands before the sibling writes.
                nc.tensor.matmul(acc_ap(accs, t),
                                 e[:, 128 * t: 128 * (t + 1)],
                                 v_sb[:, si, :],
                                 start=(si == 0 and t % 3 == 0),
                                 stop=(si == NS - 1))

        def b_evac(slot_idx):
            h, th = SLOTS[slot_idx]
            accs = baccs.pop(slot_idx)
            for t in range(NTT):
                ap = acc_ap(accs, t)
                rsl = slice(th * TBH + 128 * t, th * TBH + 128 * (t + 1))
                o_sb = osb.tile([128, OW], mybir.dt.float32, tag="o",
                                name="o_sb")
                if normalize == "dve":
                    linv = osb.tile([128, 1], mybir.dt.float32, tag="linv",
                                    name="linv")
                    nc.vector.reciprocal(linv, ap[:, 128:129])
                    nc.vector.tensor_scalar_mul(o_sb, ap[:, 0:128], linv)
                else:
                    nc.vector.tensor_copy(o_sb, ap)
                nc.sync.dma_start(out=o[h][rsl, :], in_=o_sb)

        prev = None      # slot_idx with pending B work
        for slot_idx, (h, th) in enumerate(SLOTS):
            if th == 0:
                qT_sb = qk.tile([H, T], f32r, tag="q", name="qT_sb")
                nc.sync.dma_start(out=qT_sb, in_=qT[h])
                kT_sb = qk.tile([H, S], f32r, tag="k", name="kT_sb")
                nc.sync.dma_start(out=kT_sb, in_=kT[h])
                v_sb = vpool.tile([128, NS, H + 1], bf16, tag="v", name="v_sb")
                nc.sync.dma_start(
                    out=v_sb, in_=vone[h].rearrange("(i p) d -> p i d", p=128))
                vtiles[h % 2] = v_sb
            for si in range(NS):
                # B-work of the previous slot first: those matmuls are always
                # ready, so the in-order PE queue chews on them while mm1
                # waits for a free mm1-psum buffer (ACT drain, 2 tiles back).
                if prev is not None and not skip_b:
                    b_mms(prev, si)
                ps = ps_s.tile([128, TBH], f32, tag="ps", name="ps")
                for c in range(TBH // MM_N):
                    csl = slice(th * TBH + c * MM_N, th * TBH + (c + 1) * MM_N)
                    nc.tensor.matmul(ps[:, c * MM_N:(c + 1) * MM_N],
                                     kT_sb[:, si * 128:(si + 1) * 128],
                                     qT_sb[:, csl], start=True, stop=True)
                msl = mask_sb[:, si, th * TBH:(th + 1) * TBH]
                if skip_exp:
                    etiles[(slot_idx % 2, si)] = const_e
                elif si in dve_sis:
                    # q is pre-scaled by SCALE on host, so ps holds scaled
                    # logits; the DVE ladder computes exp(ps)*mask in two
                    # fused custom instructions (no ACT involvement).
                    x_sb = xpool.tile([128, TBH], f32, tag="x", name="x_sb")
                    nc.vector._custom_dve(dveops["seed"], out=x_sb, in0=ps,
                                          s0=1.0 / 32.0, s1=1.0 / 6.0,
                                          imm2=0.5)
                    e = epool.tile([128, TBH], bf16, tag="e", name="e")
                    nc.vector._custom_dve(dveops["fin"], out=e, in0=x_sb,
                                          in1=msl)
                    etiles[(slot_idx % 2, si)] = e
                else:
                    e = epool.tile([128, TBH], bf16, tag="e", name="e")
                    nc.scalar.activation(e, ps,
                                         mybir.ActivationFunctionType.Exp,
                                         scale=1.0)
                    if si in pool_sis:
                        nc.gpsimd.tensor_mul(e, e, msl)
                    else:
                        nc.vector.tensor_mul(e, e, msl)
                    etiles[(slot_idx % 2, si)] = e
            if prev is not None and not skip_b:
                b_evac(prev)
            prev = slot_idx
        if not skip_b:
            for si in range(NS):
                b_mms(prev, si)
            b_evac(prev)

    nc.compile()
    return nc


import os as _os
VERSION = _os.environ.get("KERNEL_VERSION", "v5")


def _build_version(repeat=1):
    if VERSION == "v6":
        return _build_v5(repeat=repeat, dve_sis=(7, 15))
    if VERSION == "v5":
        return _build_v5(repeat=repeat)
    if VERSION == "v4":
        return _build_v4(repeat=repeat)
    return _build(repeat=repeat)


def _get_nc():
    key = ("nc", VERSION)
    if key not in _CACHE:
        _CACHE[key] = _build_version()
    return _CACHE[key]


def _build_repeat(repeat):
    key = ("rep", VERSION, repeat)
    if key not in _CACHE:
        _CACHE[key] = _build_version(repeat=repeat)
    return _CACHE[key]


def _shard_inputs_v5(q, k, v, mask):
    bf16 = ml_dtypes.bfloat16
    in_maps = []
    maskT = {}
    for b in range(B):
        maskT[b] = np.ascontiguousarray(mask[b, 0].T).astype(bf16)
    ones = np.ones((S, 1), dtype=bf16)
    for c in range(N_CORES):
        b = c // 4
        h0 = (c % 4) * HPC
        vb = v[b, h0:h0 + HPC].astype(bf16)                     # [HPC, S, H]
        vone = np.concatenate(
            [vb, np.broadcast_to(ones, (HPC, S, 1))], axis=-1)  # [HPC, S, H+1]
        in_maps.append({
            # q pre-scaled by 1/sqrt(H): mm1 psum holds final scaled logits
            "qT": np.ascontiguousarray(
                q[b, h0:h0 + HPC].transpose(0, 2, 1)).astype(np.float32)
            * np.float32(SCALE),
            "kT": np.ascontiguousarray(
                k[b, h0:h0 + HPC].transpose(0, 2, 1)).astype(np.float32),
            "vone": np.ascontiguousarray(vone),
            "mT": maskT[b],
        })
    return in_maps


def _shard_inputs(q, k, v, mask, mask_mode="pe", qk_dt="f32r"):
    if VERSION == "v5":
        return _shard_inputs_v5(q, k, v, mask)
    return _shard_inputs_v1(q, k, v, mask, mask_mode=mask_mode, qk_dt=qk_dt)


def _shard_inputs_v1(q, k, v, mask, mask_mode="pe", qk_dt="f32r"):
    bf16 = ml_dtypes.bfloat16
    in_maps = []
    maskT = {}
    for b in range(B):
        mt = np.ascontiguousarray(mask[b, 0].T)
        if mask_mode == "pe":
            # additive bias: 0 where unmasked, -1e38 where masked
            maskT[b] = np.where(mt, np.float32(0.0),
                                np.float32(-1e38)).astype(bf16)
        else:
            maskT[b] = mt.astype(bf16)
    ident = np.eye(128, dtype=bf16)
    for c in range(N_CORES):
        b = c // 4
        h0 = (c % 4) * HPC
        in_maps.append({
            "qT": np.ascontiguousarray(
                q[b, h0:h0 + HPC].transpose(0, 2, 1)).astype(
                    np.float32 if qk_dt == "f32r" else bf16),
            "kT": np.ascontiguousarray(
                k[b, h0:h0 + HPC].transpose(0, 2, 1)).astype(
                    np.float32 if qk_dt == "f32r" else bf16),
            "v": v[b, h0:h0 + HPC].astype(bf16),
            "mT": maskT[b],
            "ident": ident,
        })
    return in_maps


def kernel(q, k, v, mask):
    nc = _get_nc()
    in_maps = _shard_inputs(q, k, v, mask)
    res = run_bass_kernel_spmd(nc, in_maps, list(range(N_CORES)))
    out = np.empty((B, N, T, H), dtype=np.float32)
    for c in range(N_CORES):
        b = c // 4
        h0 = (c % 4) * HPC
        if VERSION == "v5":
            out[b, h0:h0 + HPC] = res.results[c]["o"]  # [HPC, T, H] normalized
        else:
            oT_c = res.results[c]["oT"]  # [HPC, H, T] unnormalized
            l_c = res.results[c]["l"]    # [HPC, T]
            out[b, h0:h0 + HPC] = (oT_c / l_c[:, None, :]).transpose(0, 2, 1)
    return out

